# revision 13
# baseline (speedup 1.0000x reference)
"""Dynamic directional conv (depthwise 7x7, 4 rotated gaussian kernels mixed
per-pixel by an angle-MLP softmax) on 8 trn2 NeuronCores.

Strategy
--------
Data-parallel over batch B=8: one batch image per core.

Per core, the depthwise conv runs as a "strip scheme" that packs
(kw-shift-block x h-rows) onto the matmul contraction axis: the moving
tile holds 3 w-shifted copies of a 38-row slice of the (reflect-padded)
image, so a single [114,128] stationary delivers all 4 directions x
7 kh-taps x 3 kw-taps at once. Per 4-channel group that is 4 h-strips x
3 fine-shift passes = 12 matmuls of 512 columns (vs 28 for the
per-(direction,kw) banded formulation). The per-pixel softmax weights
multiply the PSUM result on the vector engines (direction axis lives in
the PSUM partition dim), and a fifth matmul per strip contracts the
4-direction axis back out (identity-scatter stationary, col-tiled).

Host prep: reflect-pad H and W, regroup to [g 32][h 134][w 136][c 4]
fp16 so the strip-tile DMA reads 1040B-contiguous runs; output comes
back as [g][h][w][c] fp32 and is unshuffled on the host.
"""

import math

import numpy as np

import concourse.bass as bass
import concourse.tile as tile
from concourse import bacc, mybir
from concourse.tile_rust import add_dep_helper
from concourse.bass_utils import run_bass_kernel_spmd

F16 = mybir.dt.float16
F32 = mybir.dt.float32

B, C, H, W = 8, 128, 128, 128
K = 7
PAD = K // 2
NCG = C // 4  # 32 4-channel groups
N_CORES = 8

NS = 4        # h-strips per image
SD = 32       # output rows per strip
DHP = SD + 6  # input rows per strip
RKW = 3       # kw replication blocks
PIN = RKW * DHP  # 114 contraction rows
WE = 136      # padded W extent of the host layout (134 data + 2 slack)
HSTR = WE * 4          # h stride in elements of xg
GSTR = 134 * HSTR      # group stride

# consts layout: w1 (16) | b1 (8) | w2 (32) | b2 (4) | pi/2
IW1, IB1, IW2, IB2, IPI2 = 0, 16, 24, 56, 60
NCONST = 61

_cached_nc = None
DEBUG_TAPS = False


def _build_nc():
    nc = bacc.Bacc("TRN2", target_bir_lowering=False, debug=False)
    xg_d = nc.dram_tensor("xg", [NCG, 134, WE, 4], F16, kind="ExternalInput")
    ang_d = nc.dram_tensor("angle", [H, W], F32, kind="ExternalInput")
    cst_d = nc.dram_tensor("consts", [NCONST], F32, kind="ExternalInput")
    sc_d = nc.dram_tensor("sconv", [3, 128, 128], F16, kind="ExternalInput")
    sr_d = nc.dram_tensor("sred", [128, 32], F16, kind="ExternalInput")
    out_d = nc.dram_tensor("out", [NCG, H, W, 4], F32, kind="ExternalOutput")
    if DEBUG_TAPS:
        dbg_xt = nc.dram_tensor("dbg_xt", [128, 130, 4], F16, kind="ExternalOutput")
        dbg_ps = nc.dram_tensor("dbg_ps", [128, 512], F32, kind="ExternalOutput")
        dbg_t = nc.dram_tensor("dbg_t", [128, 512], F16, kind="ExternalOutput")

    with tile.TileContext(nc) as tc:
        with (
            tc.tile_pool(name="single", bufs=1) as single,
            tc.tile_pool(name="psum", bufs=1, space="PSUM") as psum,
        ):
            # ---- setup loads ----
            at = single.tile([128, W], F32, tag="at")
            d_ang = nc.sync.dma_start(out=at[:], in_=ang_d.ap())
            cb = single.tile([128, NCONST], F32, tag="cb")
            nc.gpsimd.dma_start(
                out=cb[:],
                in_=bass.AP(tensor=cst_d, offset=0, ap=[[0, 128], [1, NCONST]]),
            )
            # conv stationaries [p, j, q] and reduce stationary [p, q]
            sc = single.tile([128, 3, 128], F16, tag="sc")
            d_sc = nc.scalar.dma_start(
                out=sc[:], in_=sc_d.ap().rearrange("j p q -> p j q")
            )
            sr = single.tile([128, 32], F16, tag="sr")
            d_sr = nc.scalar.dma_start(out=sr[:], in_=sr_d.ap())

            # ---- strip tiles: [(kwb 3, dh' 38) = 114, wt 130, c 4] ----
            # (DMAs are emitted inline in the main loop so the tile pool's
            # version tracking binds each conv to ITS tile write; the
            # dependency-driven scheduler still prefetches ahead.)
            xt = [
                single.tile([128, 130, 4], F16, tag=f"xt{i}", name=f"xt{i}")
                for i in range(8)
            ]
            add_dep_helper(d_sr.ins, d_sc.ins, True, "serialize scalar DMAs")

            # ---- PSUM banks ----
            pconv = [
                psum.tile([128, 512], F32, tag=f"pc{i}", name=f"pc{i}")
                for i in range(4)
            ]
            pred = [
                psum.tile([128, 512], F32, tag=f"pr{i}", name=f"pr{i}")
                for i in range(2)
            ]

            # ---- PE warmup (HAM clock gate) ----
            wrm_l = single.tile([128, 128], F16, tag="wrm_l")
            wrm_r = single.tile([128, 512], F16, tag="wrm_r")
            nc.vector.memset(wrm_l[:], 0.0)
            nc.vector.memset(wrm_r[:], 0.0)
            for wi in range(12):
                nc.tensor.matmul(
                    pconv[wi % 4][:], wrm_l[:], wrm_r[:], start=True, stop=True
                )

            # ---- per-pixel mix weights: softmax(MLP(sin2a, cos2a)) ----
            sa = single.tile([128, W], F32, tag="sa")
            s2 = single.tile([128, W], F32, tag="s2")
            c2 = single.tile([128, W], F32, tag="c2")
            Act = mybir.ActivationFunctionType
            nc.scalar.activation(sa[:], at[:], Act.Sin)  # sin(a), a in [0,pi]
            nc.scalar.activation(
                c2[:], at[:], Act.Sin, bias=cb[:, IPI2 : IPI2 + 1], scale=-1.0
            )
            nc.vector.tensor_mul(s2[:], sa[:], c2[:])
            nc.scalar.mul(out=s2[:], in_=s2[:], mul=2.0)
            nc.scalar.activation(c2[:], sa[:], Act.Square, scale=float(math.sqrt(2.0)))
            nc.vector.tensor_scalar(
                out=c2[:], in0=c2[:], scalar1=-1.0, scalar2=1.0,
                op0=mybir.AluOpType.mult, op1=mybir.AluOpType.add,
            )
            hall = single.tile([128, 8, W], F32, tag="hall")
            for j in range(8):
                nc.vector.tensor_scalar(
                    out=hall[:, j, :], in0=s2[:],
                    scalar1=cb[:, IW1 + 2 * j : IW1 + 2 * j + 1],
                    scalar2=cb[:, IB1 + j : IB1 + j + 1],
                    op0=mybir.AluOpType.mult, op1=mybir.AluOpType.add,
                )
                nc.vector.scalar_tensor_tensor(
                    out=hall[:, j, :], in0=c2[:],
                    scalar=cb[:, IW1 + 2 * j + 1 : IW1 + 2 * j + 2],
                    in1=hall[:, j, :],
                    op0=mybir.AluOpType.mult, op1=mybir.AluOpType.add,
                )
                nc.vector.tensor_scalar_max(
                    out=hall[:, j, :], in0=hall[:, j, :], scalar1=0.0
                )
            eall = single.tile([128, 4, W], F32, tag="eall")
            for d in range(4):
                nc.vector.tensor_scalar(
                    out=eall[:, d, :], in0=hall[:, 0, :],
                    scalar1=cb[:, IW2 + 8 * d : IW2 + 8 * d + 1],
                    scalar2=cb[:, IB2 + d : IB2 + d + 1],
                    op0=mybir.AluOpType.mult, op1=mybir.AluOpType.add,
                )
                for j in range(1, 8):
                    nc.vector.scalar_tensor_tensor(
                        out=eall[:, d, :], in0=hall[:, j, :],
                        scalar=cb[:, IW2 + 8 * d + j : IW2 + 8 * d + j + 1],
                        in1=eall[:, d, :],
                        op0=mybir.AluOpType.mult, op1=mybir.AluOpType.add,
                    )
                nc.scalar.activation(eall[:, d, :], eall[:, d, :], Act.Exp)
            ssum = single.tile([128, W], F32, tag="ssum")
            nc.vector.tensor_add(ssum[:], eall[:, 0, :], eall[:, 1, :])
            nc.vector.tensor_add(ssum[:], ssum[:], eall[:, 2, :])
            nc.vector.tensor_add(ssum[:], ssum[:], eall[:, 3, :])
            rs = single.tile([128, W], F32, tag="rs")
            nc.vector.reciprocal(rs[:], ssum[:])
            wall = single.tile([128, 4, W], F32, tag="wall")
            for d in range(4):
                nc.vector.tensor_mul(wall[:, d, :], eall[:, d, :], rs[:])

            # wallstrip: ws[s][(4dh+d), w, 1] <- wall[h0+dh, d, w]
            ws = []
            for s in range(NS):
                w_t = single.tile([128, W, 1], F32, tag=f"ws{s}", name=f"ws{s}")
                nc.gpsimd.dma_start(
                    out=w_t[:], in_=wall[32 * s : 32 * s + 32, :, :]
                )
                ws.append(w_t)

            # ---- main loop: conv + mix + reduce per (group, strip) ----
            tt = [
                single.tile([128, 512], F16, tag=f"tt{i}", name=f"tt{i}")
                for i in range(6)
            ]
            ot = [
                single.tile([128, 512], F32, tag=f"ot{i}", name=f"ot{i}")
                for i in range(3)
            ]
            ti = 0
            last_xdma = d_ang
            for g in range(NCG):
                p2 = pred[g % 2]
                for s in range(NS):
                    t = xt[(g * NS + s) % 8]
                    src = bass.AP(
                        tensor=xg_d,
                        offset=g * GSTR + 32 * s * HSTR,
                        ap=[[12, RKW], [HSTR, DHP], [1, 520]],
                    )
                    xd = nc.sync.dma_start(out=t[0:PIN, :, :], in_=src)
                    # serialize the sync-queue stream so early groups aren't
                    # starved by packet round-robin across queued DMAs
                    add_dep_helper(xd.ins, last_xdma.ins, True, "serialize in-DMAs")
                    last_xdma = xd
                    pc = pconv[s]
                    for j in range(3):
                        nc.tensor.matmul(
                            pc[:],
                            sc[0:PIN, j, :],
                            t[0:PIN, j : j + 128, :],
                            start=(j == 0),
                            stop=(j == 2),
                        )
                    if DEBUG_TAPS and g == 0 and s == 0:
                        dxt = single.tile([128, 130, 4], F16, tag="dxt")
                        nc.vector.tensor_copy(dxt[0:PIN], t[0:PIN])
                        nc.gpsimd.dma_start(out=dbg_xt.ap()[0:PIN], in_=dxt[0:PIN])
                        dps = single.tile([128, 512], F32, tag="dps")
                        nc.scalar.copy(dps[:], pc[:])
                        nc.gpsimd.dma_start(out=dbg_ps.ap(), in_=dps[:])
                    # per-pixel weights: d lives in the partition dim
                    T = tt[ti % 6]
                    nc.vector.tensor_mul(
                        T[:].rearrange("p (w c) -> p w c", c=4),
                        pc[:].rearrange("p (w c) -> p w c", c=4),
                        ws[s][:].broadcast_to([128, W, 4]),
                    )
                    ti += 1
                    if DEBUG_TAPS and g == 0 and s == 0:
                        dtt = single.tile([128, 512], F16, tag="dtt")
                        nc.vector.tensor_copy(dtt[:], T[:])
                        nc.gpsimd.dma_start(out=dbg_t.ap(), in_=dtt[:])
                    # contract the direction axis: psum2[dh, (w,c)] += ...
                    nc.tensor.matmul(
                        p2[32 * s : 32 * s + 32, :],
                        sr[:],
                        T[:],
                        start=True,
                        stop=True,
                        tile_position=(0, 32 * s),
                    )
                o = ot[g % 3]
                nc.scalar.copy(o[:], p2[:])
                nc.scalar.dma_start(
                    out=out_d.ap()[g].rearrange("h w c -> h (w c)"), in_=o[:]
                )


    nc.compile()
    return nc


def _build_stationaries(base_kernels: np.ndarray):
    """sconv[j, (kwb,dh'), (4dh+d)] = K_d[kh=dh'-dh, kw=j+3kwb];
    sred[(4dh+d), dh] = 1."""
    sc = np.zeros((3, 128, 128), np.float16)
    for j in range(3):
        for kwb in range(RKW):
            kw = j + 3 * kwb
            if kw > 6:
                continue
            for dh in range(SD):
                for kh in range(K):
                    for d in range(4):
                        sc[j, kwb * DHP + dh + kh, 4 * dh + d] = np.float16(
                            base_kernels[d, kh, kw]
                        )
    sr = np.zeros((128, 32), np.float16)
    for dh in range(SD):
        for d in range(4):
            sr[4 * dh + d, dh] = 1.0
    return sc, sr


# results of the last run_bass_kernel_spmd call (for test harnesses)
last_results = None


def kernel(x, angle_map, w1, b1, w2, b2, base_kernels):
    global _cached_nc, last_results
    x = np.asarray(x, np.float32)
    angle_map = np.asarray(angle_map, np.float32)
    consts = np.concatenate(
        [
            np.asarray(w1, np.float32).ravel(),
            np.asarray(b1, np.float32).ravel(),
            np.asarray(w2, np.float32).ravel(),
            np.asarray(b2, np.float32).ravel(),
            [math.pi / 2],
        ]
    ).astype(np.float32)
    sc, sr = _build_stationaries(np.asarray(base_kernels, np.float32))

    # host layout: reflect-pad both dims, group to [g][h 134][w 136][c 4] fp16
    xp = np.pad(
        x, ((0, 0), (0, 0), (PAD, PAD), (PAD, PAD)), mode="reflect"
    ).astype(np.float16)
    xg = np.zeros((B, NCG, 134, WE, 4), np.float16)
    xg[:, :, :, :134, :] = (
        xp.reshape(B, NCG, 4, 134, 134).transpose(0, 1, 3, 4, 2)
    )

    if _cached_nc is None:
        _cached_nc = _build_nc()
    nc = _cached_nc

    in_maps = [
        {
            "xg": xg[b],
            "angle": angle_map[b],
            "consts": consts,
            "sconv": sc,
            "sred": sr,
        }
        for b in range(N_CORES)
    ]
    last_results = run_bass_kernel_spmd(nc, in_maps, core_ids=list(range(N_CORES)))
    # out[g][h][w][c] -> [C, H, W]
    return np.stack(
        [
            last_results.results[b]["out"]
            .transpose(0, 3, 1, 2)
            .reshape(C, H, W)
            for b in range(N_CORES)
        ]
    )


# revision 14
# speedup vs baseline: 2.9355x; 2.9355x over previous
"""Dynamic directional conv (depthwise 7x7, 4 rotated gaussian kernels mixed
per-pixel by an angle-MLP softmax) on 8 trn2 NeuronCores.

Strategy
--------
Data-parallel over batch B=8: one batch image per core.

Per core, the depthwise conv runs as a "strip scheme" that packs
(kw-shift-block x h-rows) onto the matmul contraction axis: the moving
tile holds 3 w-shifted copies of a 38-row slice of the (reflect-padded)
image, so a single [114,128] stationary delivers all 4 directions x
7 kh-taps x 3 kw-taps at once. Per 4-channel group that is 4 h-strips x
3 fine-shift passes = 12 matmuls of 512 columns (vs 28 for the
per-(direction,kw) banded formulation). The per-pixel softmax weights
multiply the PSUM result on DVE/GPSIMD (the direction axis lives in the
PSUM partition dim), and a fourth matmul per strip contracts the
4-direction axis back out (identity-scatter stationary, col-tiled into
one PSUM bank per group).

Host prep: reflect-pad, then pre-replicate the strip tiles to
[g 32][s 4][114][520] fp16 so every input DMA is one contiguous 118KB
block; output returns as [g][h][w][c4] fp32 and is unshuffled on host.
"""

import math

import numpy as np

import concourse.bass as bass
import concourse.tile as tile
from concourse import bacc, mybir
from concourse.tile_rust import add_dep_helper
from concourse.bass_utils import run_bass_kernel_spmd

F16 = mybir.dt.float16
F32 = mybir.dt.float32

B, C, H, W = 8, 128, 128, 128
K = 7
PAD = K // 2
NCG = C // 4  # 32 4-channel groups
N_CORES = 8

NS = 4        # h-strips per image
SD = 32       # output rows per strip
DHP = SD + 6  # input rows per strip
RKW = 3       # kw replication blocks
PIN = RKW * DHP  # 114 contraction rows
TFREE = 130 * 4  # tile free size (wt 130, c 4)

# consts layout: w1 (16) | b1 (8) | w2 (32) | b2 (4) | pi/2
IW1, IB1, IW2, IB2, IPI2 = 0, 16, 24, 56, 60
NCONST = 61

_cached_nc = None


def _build_nc():
    nc = bacc.Bacc("TRN2", target_bir_lowering=False, debug=False)
    xt_d = nc.dram_tensor("xtiles", [NCG, NS, PIN, TFREE], F16, kind="ExternalInput")
    ang_d = nc.dram_tensor("angle", [H, W], F32, kind="ExternalInput")
    cst_d = nc.dram_tensor("consts", [NCONST], F32, kind="ExternalInput")
    sc_d = nc.dram_tensor("sconv", [3, 128, 128], F16, kind="ExternalInput")
    sr_d = nc.dram_tensor("sred", [128, 32], F16, kind="ExternalInput")
    out_d = nc.dram_tensor("out", [NCG, H, W, 4], F32, kind="ExternalOutput")

    with tile.TileContext(nc) as tc:
        with (
            tc.tile_pool(name="single", bufs=1) as single,
            tc.tile_pool(name="psum", bufs=1, space="PSUM") as psum,
        ):
            # ---- setup loads ----
            at = single.tile([128, W], F32, tag="at")
            nc.sync.dma_start(out=at[:], in_=ang_d.ap())
            cb = single.tile([128, NCONST], F32, tag="cb")
            nc.gpsimd.dma_start(
                out=cb[:],
                in_=bass.AP(tensor=cst_d, offset=0, ap=[[0, 128], [1, NCONST]]),
            )
            # conv stationaries [p, j, q] and reduce stationary [p, q]
            sc = single.tile([128, 3, 128], F16, tag="sc")
            d_sc = nc.scalar.dma_start(
                out=sc[:], in_=sc_d.ap().rearrange("j p q -> p j q")
            )
            sr = single.tile([128, 32], F16, tag="sr")
            d_sr = nc.scalar.dma_start(out=sr[:], in_=sr_d.ap())
            add_dep_helper(d_sr.ins, d_sc.ins, True, "serialize scalar DMAs")

            # strip tiles (DMA'd inline in the main loop; scheduler prefetches)
            xt = [
                single.tile([128, TFREE], F16, tag=f"xt{i}", name=f"xt{i}")
                for i in range(12)
            ]

            # ---- PSUM banks ----
            pconv = [
                psum.tile([128, 512], F32, tag=f"pc{i}", name=f"pc{i}")
                for i in range(4)
            ]
            pred = [
                psum.tile([128, 512], F32, tag=f"pr{i}", name=f"pr{i}")
                for i in range(2)
            ]

            # ---- PE warmup (HAM clock gate) ----
            wrm_l = single.tile([128, 128], F16, tag="wrm_l")
            wrm_r = single.tile([128, 512], F16, tag="wrm_r")
            nc.vector.memset(wrm_l[:], 0.0)
            nc.vector.memset(wrm_r[:], 0.0)
            for wi in range(12):
                nc.tensor.matmul(
                    pconv[wi % 4][:], wrm_l[:], wrm_r[:], start=True, stop=True
                )

            # ---- per-pixel mix weights: softmax(MLP(sin2a, cos2a)) ----
            sa = single.tile([128, W], F32, tag="sa")
            s2 = single.tile([128, W], F32, tag="s2")
            c2 = single.tile([128, W], F32, tag="c2")
            Act = mybir.ActivationFunctionType
            nc.scalar.activation(sa[:], at[:], Act.Sin)  # sin(a), a in [0,pi]
            nc.scalar.activation(
                c2[:], at[:], Act.Sin, bias=cb[:, IPI2 : IPI2 + 1], scale=-1.0
            )
            nc.vector.tensor_mul(s2[:], sa[:], c2[:])
            nc.scalar.mul(out=s2[:], in_=s2[:], mul=2.0)
            nc.scalar.activation(c2[:], sa[:], Act.Square, scale=float(math.sqrt(2.0)))
            nc.vector.tensor_scalar(
                out=c2[:], in0=c2[:], scalar1=-1.0, scalar2=1.0,
                op0=mybir.AluOpType.mult, op1=mybir.AluOpType.add,
            )
            hall = single.tile([128, 8, W], F32, tag="hall")
            for j in range(8):
                nc.vector.tensor_scalar(
                    out=hall[:, j, :], in0=s2[:],
                    scalar1=cb[:, IW1 + 2 * j : IW1 + 2 * j + 1],
                    scalar2=cb[:, IB1 + j : IB1 + j + 1],
                    op0=mybir.AluOpType.mult, op1=mybir.AluOpType.add,
                )
                nc.vector.scalar_tensor_tensor(
                    out=hall[:, j, :], in0=c2[:],
                    scalar=cb[:, IW1 + 2 * j + 1 : IW1 + 2 * j + 2],
                    in1=hall[:, j, :],
                    op0=mybir.AluOpType.mult, op1=mybir.AluOpType.add,
                )
                nc.vector.tensor_scalar_max(
                    out=hall[:, j, :], in0=hall[:, j, :], scalar1=0.0
                )
            eall = single.tile([128, 4, W], F32, tag="eall")
            for d in range(4):
                nc.vector.tensor_scalar(
                    out=eall[:, d, :], in0=hall[:, 0, :],
                    scalar1=cb[:, IW2 + 8 * d : IW2 + 8 * d + 1],
                    scalar2=cb[:, IB2 + d : IB2 + d + 1],
                    op0=mybir.AluOpType.mult, op1=mybir.AluOpType.add,
                )
                for j in range(1, 8):
                    nc.vector.scalar_tensor_tensor(
                        out=eall[:, d, :], in0=hall[:, j, :],
                        scalar=cb[:, IW2 + 8 * d + j : IW2 + 8 * d + j + 1],
                        in1=eall[:, d, :],
                        op0=mybir.AluOpType.mult, op1=mybir.AluOpType.add,
                    )
                nc.scalar.activation(eall[:, d, :], eall[:, d, :], Act.Exp)
            ssum = single.tile([128, W], F32, tag="ssum")
            nc.vector.tensor_add(ssum[:], eall[:, 0, :], eall[:, 1, :])
            nc.vector.tensor_add(ssum[:], ssum[:], eall[:, 2, :])
            nc.vector.tensor_add(ssum[:], ssum[:], eall[:, 3, :])
            rs = single.tile([128, W], F32, tag="rs")
            nc.vector.reciprocal(rs[:], ssum[:])
            wall = single.tile([128, 4, W], F32, tag="wall")
            for d in range(4):
                nc.vector.tensor_mul(wall[:, d, :], eall[:, d, :], rs[:])

            # wallstrip: ws[s][(4dh+d), w, 1] <- wall[h0+dh, d, w]
            ws = []
            for s in range(NS):
                w_t = single.tile([128, W, 1], F32, tag=f"ws{s}", name=f"ws{s}")
                nc.gpsimd.dma_start(
                    out=w_t[:], in_=wall[32 * s : 32 * s + 32, :, :]
                )
                ws.append(w_t)

            # ---- main loop: conv + mix + reduce per (group, strip) ----
            tt = [
                single.tile([128, 512], F16, tag=f"tt{i}", name=f"tt{i}")
                for i in range(6)
            ]
            pcs = [
                single.tile([128, 512], F32, tag=f"pcs{i}", name=f"pcs{i}")
                for i in range(4)
            ]
            ot = [
                single.tile([128, 512], F32, tag=f"ot{i}", name=f"ot{i}")
                for i in range(3)
            ]
            ti = 0
            ci = 0
            for g in range(NCG):
                p2 = pred[g % 2]
                for s in range(NS):
                    t = xt[(g * NS + s) % 12]
                    # input stream alternates the two HWDGE queues
                    deng = nc.sync if (g * NS + s) % 2 == 0 else nc.scalar
                    deng.dma_start(out=t[0:PIN, :], in_=xt_d.ap()[g, s])
                    pc = pconv[s]
                    for j in range(3):
                        nc.tensor.matmul(
                            pc[:],
                            sc[0:PIN, j, :],
                            t[0:PIN, 4 * j : 4 * j + 512],
                            start=(j == 0),
                            stop=(j == 2),
                        )
                    # apply per-pixel weights (direction axis in partitions):
                    # DVE reads PSUM directly; for half the strips route via
                    # an ACT psum->sbuf copy + GPSIMD mul to spread load
                    T = tt[ti % 6]
                    wsb = ws[s][:].broadcast_to([128, W, 4])
                    tv = T[:].rearrange("p (w c) -> p w c", c=4)
                    if s % 2 == 0:
                        nc.vector.tensor_mul(
                            tv, pc[:].rearrange("p (w c) -> p w c", c=4), wsb
                        )
                    else:
                        pp = pcs[ci % 4]
                        ci += 1
                        nc.scalar.copy(pp[:], pc[:])
                        nc.gpsimd.tensor_mul(
                            tv, pp[:].rearrange("p (w c) -> p w c", c=4), wsb
                        )
                    ti += 1
                    # contract the direction axis: psum2[dh, (w,c)]
                    nc.tensor.matmul(
                        p2[32 * s : 32 * s + 32, :],
                        sr[:],
                        T[:],
                        start=True,
                        stop=True,
                        tile_position=(0, 32 * s),
                    )
                o = ot[g % 3]
                nc.scalar.copy(o[:], p2[:])
                nc.gpsimd.dma_start(
                    out=out_d.ap()[g].rearrange("h w c -> h (w c)"), in_=o[:]
                )

    nc.compile()
    return nc


def _build_stationaries(base_kernels: np.ndarray):
    """sconv[j, (kwb,dh'), (4dh+d)] = K_d[kh=dh'-dh, kw=j+3kwb];
    sred[(4dh+d), dh] = 1."""
    sc = np.zeros((3, 128, 128), np.float16)
    for j in range(3):
        for kwb in range(RKW):
            kw = j + 3 * kwb
            if kw > 6:
                continue
            for dh in range(SD):
                for kh in range(K):
                    for d in range(4):
                        sc[j, kwb * DHP + dh + kh, 4 * dh + d] = np.float16(
                            base_kernels[d, kh, kw]
                        )
    sr = np.zeros((128, 32), np.float16)
    for dh in range(SD):
        for d in range(4):
            sr[4 * dh + d, dh] = 1.0
    return sc, sr


def _build_xtiles(xb: np.ndarray) -> np.ndarray:
    """Pre-replicated strip tiles [g, s, (kwb,dh'), (wt,c)] fp16 from one
    batch image [C, H, W] fp32."""
    xp = np.zeros((C, 134, 136), np.float16)
    xp[:, :, :134] = np.pad(xb, ((0, 0), (3, 3), (3, 3)), mode="reflect")
    xpg = xp.reshape(NCG, 4, 134, 136)
    t = np.empty((NCG, NS, RKW, DHP, 130, 4), np.float16)
    for s in range(NS):
        rows = xpg[:, :, 32 * s : 32 * s + DHP, :]  # (g, c, 38, 136)
        for kwb in range(RKW):
            # (g, c, 38, 130) -> (g, 38, 130, c)
            t[:, s, kwb] = rows[:, :, :, 3 * kwb : 3 * kwb + 130].transpose(
                0, 2, 3, 1
            )
    return t.reshape(NCG, NS, PIN, TFREE)


# results of the last run_bass_kernel_spmd call (for test harnesses)
last_results = None


def kernel(x, angle_map, w1, b1, w2, b2, base_kernels):
    global _cached_nc, last_results
    x = np.asarray(x, np.float32)
    angle_map = np.asarray(angle_map, np.float32)
    consts = np.concatenate(
        [
            np.asarray(w1, np.float32).ravel(),
            np.asarray(b1, np.float32).ravel(),
            np.asarray(w2, np.float32).ravel(),
            np.asarray(b2, np.float32).ravel(),
            [math.pi / 2],
        ]
    ).astype(np.float32)
    sc, sr = _build_stationaries(np.asarray(base_kernels, np.float32))

    if _cached_nc is None:
        _cached_nc = _build_nc()
    nc = _cached_nc

    in_maps = [
        {
            "xtiles": _build_xtiles(x[b]),
            "angle": angle_map[b],
            "consts": consts,
            "sconv": sc,
            "sred": sr,
        }
        for b in range(N_CORES)
    ]
    last_results = run_bass_kernel_spmd(nc, in_maps, core_ids=list(range(N_CORES)))
    # out[g][h][w][c] -> [C, H, W]
    return np.stack(
        [
            last_results.results[b]["out"]
            .transpose(0, 3, 1, 2)
            .reshape(C, H, W)
            for b in range(N_CORES)
        ]
    )
